# revision 28
# baseline (speedup 1.0000x reference)
"""Trainium2 Bass kernel for CellPathwayAttentionAggregator (segment-reduce).

Math: out[b, s] = sum_{i in set s} softmax_s(attn_logits)[i] * G[b, flat_idx[i]]

Device decomposition (per core):
    out = (G @ W_exp) * (1 / denom)[None, :]
where W_exp[g, s] = sum_{i in set s, flat_idx[i]=g} exp(attn_logits[i]) is the
(unnormalized) sparse aggregation matrix, scattered on the host as pure layout
prep (elementwise exp + scatter; no reductions on host), and
    denom[s] = sum_{i in set s} exp(attn_logits[i])
is computed ON DEVICE from a 128-slot padded logits tile (ACT exp + ones-vector
matmul), followed by on-device normalization of the matmul output.

Sharding: 8 cores = 2 batch groups (512 rows) x 4 set groups (512 sets).
Each core runs a (512 x 8192) @ (8192 x 512) bf16 matmul accumulated in fp32
PSUM over 64 K-tiles, then scales each output column by 1/denom.
"""

import sys

if "/opt/trn_rl_repo" not in sys.path:
    sys.path.insert(0, "/opt/trn_rl_repo")

import ml_dtypes
import numpy as np

NUM_SETS = 2048
NUM_GENESETS = 8192
BATCH = 1024
N_CORES = 8
BG, SG = 2, 4  # batch groups x set groups (BG*SG == N_CORES)
B_C = BATCH // BG  # 512 batch rows per core
S_C = NUM_SETS // SG  # 512 sets per core
P = 128
K_TILES = NUM_GENESETS // P  # 64
M_TILES = B_C // P  # 4
PAD_SLOTS = 128  # >= MAX set size (120)
NEG_FILL = -87.0  # exp(-87) ~ 1.6e-38 ~ 0 in fp32

_PROGRAM_CACHE = {}
LAST_RESULTS = None  # BassKernelResults of the most recent run (for profiling)


def _build_program():
    import concourse.mybir as mybir
    from concourse import bacc
    from concourse.tile import TileContext

    f32 = mybir.dt.float32
    bf16 = mybir.dt.bfloat16

    nc = bacc.Bacc("TRN2", target_bir_lowering=False, debug=False)
    # fused per-K-tile input: [:, :, :B_C] = G^T tile, [:, :, B_C:] = W tile.
    # One DMA per K-tile keeps every matmul's sync-wait count at <=1 (the
    # S3 LDWEIGHTS encoding only has a single wait slot).
    gw_d = nc.dram_tensor("gw", [K_TILES, P, B_C + S_C], bf16, kind="ExternalInput")
    plog_d = nc.dram_tensor("plog", [PAD_SLOTS, S_C], f32, kind="ExternalInput")
    out_d = nc.dram_tensor("out", [B_C, S_C], f32, kind="ExternalOutput")

    with TileContext(nc) as tc:
        with (
            tc.tile_pool(name="const", bufs=1) as cpool,
            tc.tile_pool(name="gw", bufs=16) as gwpool,
            tc.tile_pool(name="outp", bufs=4) as opool,
            tc.tile_pool(name="ps", bufs=1, space="PSUM") as ppool,
        ):
            # --- inputs for the denominator chain (SWDGE so it doesn't queue
            # behind the gw HWDGE stream) ---
            plog_sb = cpool.tile([PAD_SLOTS, S_C], f32, tag="plog")
            nc.gpsimd.dma_start(out=plog_sb[:], in_=plog_d[:, :])
            exp_sb = cpool.tile([PAD_SLOTS, S_C], f32, tag="exp")
            nc.scalar.activation(
                exp_sb[:], plog_sb[:], mybir.ActivationFunctionType.Exp
            )
            # ones vector built on ACT so the denom matmul waits on one engine
            ones_col = cpool.tile([P, 1], f32, tag="onec")
            nc.scalar.activation(
                ones_col[:],
                plog_sb[:, 0:1],
                mybir.ActivationFunctionType.Copy,
                bias=1.0,
                scale=0.0,
            )
            ones_row = cpool.tile([1, P], f32, tag="oner")
            nc.vector.memset(ones_row[:], 1.0)

            # --- main matmul: out = G_c @ W_c, accumulated over 64 K-tiles ---
            acc = [
                ppool.tile([P, S_C], f32, tag=f"acc{m}", name=f"acc{m}")
                for m in range(M_TILES)
            ]
            denom_ps = ppool.tile([1, S_C], f32, tag="denom")
            recip_sb = cpool.tile([1, S_C], f32, tag="recip")
            rep_ps = ppool.tile([P, S_C], f32, tag="rep")
            recip_rep = cpool.tile([P, S_C], f32, tag="recrep")
            for k in range(K_TILES):
                gw_sb = gwpool.tile([P, B_C + S_C], bf16, tag="gw")
                nc.sync.dma_start(out=gw_sb[:], in_=gw_d[k, :, :])
                for m in range(M_TILES):
                    nc.tensor.matmul(
                        acc[m][:],
                        gw_sb[:, m * P : (m + 1) * P],
                        gw_sb[:, B_C : B_C + S_C],
                        start=(k == 0),
                        stop=(k == K_TILES - 1),
                    )
                # denominator + replication chain injected mid-stream so the
                # reciprocal is ready long before the epilogue
                if k == 8:
                    nc.tensor.matmul(
                        denom_ps[:], ones_col[:], exp_sb[:], start=True, stop=True
                    )
                elif k == 9:
                    nc.vector.reciprocal(recip_sb[:], denom_ps[:])
                elif k == 12:
                    nc.tensor.matmul(
                        rep_ps[:], ones_row[:], recip_sb[:], start=True, stop=True
                    )
                elif k == 13:
                    nc.vector.tensor_copy(recip_rep[:], rep_ps[:])

            # --- normalize each output column by 1/denom and store ---
            for m in range(M_TILES):
                o_sb = opool.tile([P, S_C], f32, tag="osb")
                nc.vector.tensor_mul(o_sb[:], acc[m][:], recip_rep[:])
                nc.sync.dma_start(out=out_d[m * P : (m + 1) * P, :], in_=o_sb[:])

    nc.finalize()
    return nc


def _build_program_raw():
    """Raw-Bass pipeline with hand-placed semaphores — avoids the Tile/Bacc
    event-semaphore preamble (~7us) and exit butterfly (~8us).

    Sem plan (each instruction carries at most one attached wait):
      s_dma:  +16 per input DMA on Sync (plog first, then gw tiles k=0..63)
      s_mm:   +1 by PE after finishing the 4 matmuls of gw tile k
      s_init: +1 by DVE after the zero/ones memsets (gates ACT + rep matmul)
      s_act:  +1 by ACT when exp tile + ones column are ready
      s_den:  +1 by PE after the denominator matmul (gates reciprocal)
      s_dve:  +1 by DVE after the reciprocal (gates rep matmul)
      s_rep:  +1 by PE after the rep matmul (gates recip_rep copy)
      s_out:  +1 by DVE per normalized output tile (gates out DMA)
      s_done: +16 per out DMA (final drain wait)
    """
    import concourse.bass as bass
    import concourse.mybir as mybir

    f32 = mybir.dt.float32
    bf16 = mybir.dt.bfloat16
    FD = B_C + S_C  # fused free dim: 1024
    BUFS = 10

    nc = bass.Bass()
    gw_d = nc.dram_tensor("gw", [K_TILES, P, FD], bf16, kind="ExternalInput")
    plog_d = nc.dram_tensor("plog", [PAD_SLOTS, S_C], f32, kind="ExternalInput")
    out_d = nc.dram_tensor("out", [B_C, S_C], f32, kind="ExternalOutput")

    from contextlib import ExitStack

    with ExitStack() as ctx:
        gw_sb = ctx.enter_context(nc.sbuf_tensor([P, BUFS, FD], bf16))
        plog_sb = ctx.enter_context(nc.sbuf_tensor([PAD_SLOTS, S_C], f32))
        exp_sb = ctx.enter_context(nc.sbuf_tensor([PAD_SLOTS, S_C], f32))
        zero_col = ctx.enter_context(nc.sbuf_tensor([P, 1], f32))
        ones_col = ctx.enter_context(nc.sbuf_tensor([P, 1], f32))
        ones_row = ctx.enter_context(nc.sbuf_tensor([1, P], f32))
        recip_sb = ctx.enter_context(nc.sbuf_tensor([1, S_C], f32))
        recip_rep = ctx.enter_context(nc.sbuf_tensor([P, S_C], f32))
        o_sb = ctx.enter_context(nc.sbuf_tensor([P, M_TILES, S_C], f32))
        acc_ps = ctx.enter_context(nc.psum_tensor([P, M_TILES, S_C], f32))
        denom_ps = ctx.enter_context(nc.psum_tensor([1, S_C], f32))
        rep_ps = ctx.enter_context(nc.psum_tensor([P, S_C], f32))
        s_slot = [ctx.enter_context(nc.semaphore(name=f"s_slot{j}")) for j in range(BUFS)]
        s_plog = ctx.enter_context(nc.semaphore())
        s_mm = ctx.enter_context(nc.semaphore())
        s_init = ctx.enter_context(nc.semaphore())
        s_act = ctx.enter_context(nc.semaphore())
        s_den = ctx.enter_context(nc.semaphore())
        s_dve = ctx.enter_context(nc.semaphore())
        s_rep = ctx.enter_context(nc.semaphore())
        s_out = ctx.enter_context(nc.semaphore())
        s_done = ctx.enter_context(nc.semaphore())
        s_fin = ctx.enter_context(nc.semaphore())
        block = ctx.enter_context(nc.Block())

        @block.sync
        def _(sync):
            sync.dma_start(plog_sb[:], plog_d[:, :]).then_inc(s_plog, 16)
            for k in range(K_TILES):
                if k >= BUFS:
                    sync.wait_ge(s_mm, k - BUFS + 1)
                sync.dma_start(gw_sb[:, k % BUFS, :], gw_d[k, :, :]).then_inc(
                    s_slot[k % BUFS], 16
                )
            for m in range(M_TILES):
                sync.wait_ge(s_out, m + 1)
                sync.dma_start(
                    out_d[m * P : (m + 1) * P, :], o_sb[:, m, :]
                ).then_inc(s_done, 16)
            sync.wait_ge(s_done, 16 * M_TILES)

        @block.scalar
        def _(scalar):
            scalar.wait_ge(s_init, 1)
            scalar.wait_ge(s_plog, 16)
            scalar.activation(
                exp_sb[:],
                plog_sb[:],
                mybir.ActivationFunctionType.Exp,
                bias=zero_col[:],
            )
            scalar.activation(
                ones_col[:],
                plog_sb[:, 0:1],
                mybir.ActivationFunctionType.Copy,
                bias=1.0,
                scale=0.0,
            ).then_inc(s_act, 1)

        @block.tensor
        def _(tensor):
            for k in range(K_TILES):
                tensor.wait_ge(s_slot[k % BUFS], 16 * (k // BUFS + 1))
                tile = gw_sb[:, k % BUFS, :]
                for m in range(M_TILES):
                    mm = tensor.matmul(
                        acc_ps[:, m, :],
                        tile[:, m * P : (m + 1) * P],
                        tile[:, B_C:FD],
                        start=(k == 0),
                        stop=(k == K_TILES - 1),
                    )
                    if m == M_TILES - 1:
                        # rhs/lhsT fully streamed at retire -> safe to reuse
                        # the SBUF slot (write-back handled by drains below)
                        mm.then_inc(s_mm, 1)
                if k == 8:
                    tensor.wait_ge(s_act, 1)
                    tensor.matmul(
                        denom_ps[:], ones_col[:], exp_sb[:], start=True, stop=True
                    )
                    # drain flushes the PSUM writeback before consumers read
                    tensor.drain().then_inc(s_den, 1)
                elif k == 16:
                    tensor.wait_ge(s_dve, 1)
                    tensor.matmul(
                        rep_ps[:], ones_row[:], recip_sb[:], start=True, stop=True
                    )
                    tensor.drain().then_inc(s_rep, 1)
            tensor.drain().then_inc(s_fin, 1)

        @block.vector
        def _(vector):
            vector.memset(zero_col[:], 0.0)
            vector.memset(ones_row[:], 1.0).then_inc(s_init, 1)
            vector.wait_ge(s_den, 1)
            nc.vector.reciprocal(recip_sb[:], denom_ps[:]).then_inc(s_dve, 1)
            vector.wait_ge(s_rep, 1)
            nc.vector.tensor_copy(recip_rep[:], rep_ps[:])
            vector.wait_ge(s_fin, 1)
            for m in range(M_TILES):
                nc.vector.tensor_mul(
                    o_sb[:, m, :], acc_ps[:, m, :], recip_rep[:]
                ).then_inc(s_out, 1)

    nc.finalize()
    return nc


def _get_program():
    if "nc" not in _PROGRAM_CACHE:
        _PROGRAM_CACHE["nc"] = _build_program()
    return _PROGRAM_CACHE["nc"]


def _ensure_ntff_hook():
    """Make NTFF profiling under axon work (BASS_TRACE=1): the image's antenv
    package lacks the axon_hooks holder module, so synthesize it and register
    the ctypes-based profile hook from trn_agent_boot. Best-effort."""
    import types

    try:
        import antenv

        try:
            from antenv.axon_hooks import get_axon_ntff_profile_hook  # noqa: F401

            return  # already present and registered
        except ImportError:
            pass
        mod = types.ModuleType("antenv.axon_hooks")
        _holder = [None]
        mod.set_axon_ntff_profile_hook = lambda h: _holder.__setitem__(0, h)
        mod.get_axon_ntff_profile_hook = lambda: _holder[0]
        sys.modules["antenv.axon_hooks"] = mod
        antenv.axon_hooks = mod

        from trn_agent_boot.trn_boot import _ntff_profile_via_ctypes

        hook = _ntff_profile_via_ctypes("/opt/axon/libaxon_pjrt.so")
        mod.set_axon_ntff_profile_hook(hook)
    except Exception:
        pass


def kernel(**inputs):
    global LAST_RESULTS
    G = np.asarray(inputs["geneset_features"], dtype=np.float32)
    logits = np.asarray(inputs["attn_logits"], dtype=np.float32)
    flat_idx = np.asarray(inputs["flat_idx"]).astype(np.int64)
    seg = np.asarray(inputs["segment_ids"]).astype(np.int64)
    T = logits.shape[0]

    # Host-side layout prep: scatter exp(logits) into the sparse aggregation
    # matrix (member sets are sampled without replacement, so (idx, seg) pairs
    # are unique within a set and the fancy assignment is collision-free).
    e32 = np.exp(logits)
    W = np.zeros((NUM_GENESETS, NUM_SETS), dtype=ml_dtypes.bfloat16)
    W[flat_idx, seg] = e32.astype(ml_dtypes.bfloat16)

    # Padded per-set logit columns; device computes denominators from these.
    sizes = np.bincount(seg, minlength=NUM_SETS)
    starts = np.concatenate([[0], np.cumsum(sizes)[:-1]])
    pos = np.arange(T) - starts[seg]
    plogT = np.full((PAD_SLOTS, NUM_SETS), NEG_FILL, dtype=np.float32)
    plogT[pos, seg] = logits

    Gb = G.astype(ml_dtypes.bfloat16)

    GbT = np.ascontiguousarray(Gb.T)  # (8192, 1024)
    in_maps = []
    for c in range(N_CORES):
        bg, sg = divmod(c, SG)
        gt = GbT[:, bg * B_C : (bg + 1) * B_C].reshape(K_TILES, P, B_C)
        w = W[:, sg * S_C : (sg + 1) * S_C].reshape(K_TILES, P, S_C)
        gw = np.concatenate([gt, w], axis=2)  # (K_TILES, P, B_C + S_C)
        plog = np.ascontiguousarray(plogT[:, sg * S_C : (sg + 1) * S_C])
        in_maps.append({"gw": np.ascontiguousarray(gw), "plog": plog})

    from concourse.bass_utils import run_bass_kernel_spmd

    _ensure_ntff_hook()
    nc = _get_program()
    res = run_bass_kernel_spmd(nc, in_maps, core_ids=list(range(N_CORES)))
    LAST_RESULTS = res

    out = np.empty((BATCH, NUM_SETS), dtype=np.float32)
    for c in range(N_CORES):
        bg, sg = divmod(c, SG)
        out[bg * B_C : (bg + 1) * B_C, sg * S_C : (sg + 1) * S_C] = res.results[c][
            "out"
        ]
    return out


# revision 29
# speedup vs baseline: 1.0160x; 1.0160x over previous
"""Trainium2 Bass kernel for CellPathwayAttentionAggregator (segment-reduce).

Math: out[b, s] = sum_{i in set s} softmax_s(attn_logits)[i] * G[b, flat_idx[i]]

Device decomposition (per core):
    out = (G @ W_exp) * (1 / denom)[None, :]
where W_exp[g, s] = sum_{i in set s, flat_idx[i]=g} exp(attn_logits[i]) is the
(unnormalized) sparse aggregation matrix, scattered on the host as pure layout
prep (elementwise exp + scatter; no reductions on host), and
    denom[s] = sum_{i in set s} exp(attn_logits[i])
is computed ON DEVICE from a 128-slot padded logits tile (ACT exp + ones-vector
matmul), followed by on-device normalization of the matmul output.

Sharding: 8 cores = 2 batch groups (512 rows) x 4 set groups (512 sets).
Each core runs a (512 x 8192) @ (8192 x 512) bf16 matmul accumulated in fp32
PSUM over 64 K-tiles, then scales each output column by 1/denom.
"""

import sys

if "/opt/trn_rl_repo" not in sys.path:
    sys.path.insert(0, "/opt/trn_rl_repo")

import ml_dtypes
import numpy as np

NUM_SETS = 2048
NUM_GENESETS = 8192
BATCH = 1024
N_CORES = 8
BG, SG = 2, 4  # batch groups x set groups (BG*SG == N_CORES)
B_C = BATCH // BG  # 512 batch rows per core
S_C = NUM_SETS // SG  # 512 sets per core
P = 128
K_TILES = NUM_GENESETS // P  # 64
M_TILES = B_C // P  # 4
PAD_SLOTS = 128  # >= MAX set size (120)
NEG_FILL = -87.0  # exp(-87) ~ 1.6e-38 ~ 0 in fp32

_PROGRAM_CACHE = {}
LAST_RESULTS = None  # BassKernelResults of the most recent run (for profiling)


def _build_program():
    import concourse.mybir as mybir
    from concourse import bacc
    from concourse.tile import TileContext

    f32 = mybir.dt.float32
    bf16 = mybir.dt.bfloat16

    nc = bacc.Bacc("TRN2", target_bir_lowering=False, debug=False)
    # fused per-K-tile input: [:, :, :B_C] = G^T tile, [:, :, B_C:] = W tile.
    # One DMA per K-tile keeps every matmul's sync-wait count at <=1 (the
    # S3 LDWEIGHTS encoding only has a single wait slot).
    gw_d = nc.dram_tensor("gw", [K_TILES, P, B_C + S_C], bf16, kind="ExternalInput")
    plog_d = nc.dram_tensor("plog", [PAD_SLOTS, S_C], f32, kind="ExternalInput")
    out_d = nc.dram_tensor("out", [B_C, S_C], f32, kind="ExternalOutput")

    with TileContext(nc) as tc:
        with (
            tc.tile_pool(name="const", bufs=1) as cpool,
            tc.tile_pool(name="gw", bufs=12) as gwpool,
            tc.tile_pool(name="outp", bufs=4) as opool,
            tc.tile_pool(name="ps", bufs=1, space="PSUM") as ppool,
        ):
            # --- inputs for the denominator chain (SWDGE so it doesn't queue
            # behind the gw HWDGE stream) ---
            plog_sb = cpool.tile([PAD_SLOTS, S_C], f32, tag="plog")
            nc.gpsimd.dma_start(out=plog_sb[:], in_=plog_d[:, :])
            exp_sb = cpool.tile([PAD_SLOTS, S_C], f32, tag="exp")
            nc.scalar.activation(
                exp_sb[:], plog_sb[:], mybir.ActivationFunctionType.Exp
            )
            # ones vector built on ACT so the denom matmul waits on one engine
            ones_col = cpool.tile([P, 1], f32, tag="onec")
            nc.scalar.activation(
                ones_col[:],
                plog_sb[:, 0:1],
                mybir.ActivationFunctionType.Copy,
                bias=1.0,
                scale=0.0,
            )
            ones_row = cpool.tile([1, P], f32, tag="oner")
            nc.vector.memset(ones_row[:], 1.0)

            # --- main matmul: out = G_c @ W_c, accumulated over 64 K-tiles ---
            acc = [
                ppool.tile([P, S_C], f32, tag=f"acc{m}", name=f"acc{m}")
                for m in range(M_TILES)
            ]
            denom_ps = ppool.tile([1, S_C], f32, tag="denom")
            recip_sb = cpool.tile([1, S_C], f32, tag="recip")
            rep_ps = ppool.tile([P, S_C], f32, tag="rep")
            recip_rep = cpool.tile([P, S_C], f32, tag="recrep")
            for k in range(K_TILES):
                gw_sb = gwpool.tile([P, B_C + S_C], bf16, tag="gw")
                nc.sync.dma_start(out=gw_sb[:], in_=gw_d[k, :, :])
                for m in range(M_TILES):
                    nc.tensor.matmul(
                        acc[m][:],
                        gw_sb[:, m * P : (m + 1) * P],
                        gw_sb[:, B_C : B_C + S_C],
                        start=(k == 0),
                        stop=(k == K_TILES - 1),
                    )
                # denominator + replication chain injected mid-stream so the
                # reciprocal is ready long before the epilogue
                if k == 8:
                    nc.tensor.matmul(
                        denom_ps[:], ones_col[:], exp_sb[:], start=True, stop=True
                    )
                elif k == 9:
                    nc.vector.reciprocal(recip_sb[:], denom_ps[:])
                elif k == 12:
                    nc.tensor.matmul(
                        rep_ps[:], ones_row[:], recip_sb[:], start=True, stop=True
                    )
                elif k == 13:
                    nc.vector.tensor_copy(recip_rep[:], rep_ps[:])

            # --- normalize each output column by 1/denom and store ---
            for m in range(M_TILES):
                o_sb = opool.tile([P, S_C], f32, tag="osb")
                nc.vector.tensor_mul(o_sb[:], acc[m][:], recip_rep[:])
                nc.sync.dma_start(out=out_d[m * P : (m + 1) * P, :], in_=o_sb[:])

    nc.finalize()
    return nc


def _build_program_raw():
    """Raw-Bass pipeline with hand-placed semaphores — avoids the Tile/Bacc
    event-semaphore preamble (~7us) and exit butterfly (~8us).

    Sem plan (each instruction carries at most one attached wait):
      s_dma:  +16 per input DMA on Sync (plog first, then gw tiles k=0..63)
      s_mm:   +1 by PE after finishing the 4 matmuls of gw tile k
      s_init: +1 by DVE after the zero/ones memsets (gates ACT + rep matmul)
      s_act:  +1 by ACT when exp tile + ones column are ready
      s_den:  +1 by PE after the denominator matmul (gates reciprocal)
      s_dve:  +1 by DVE after the reciprocal (gates rep matmul)
      s_rep:  +1 by PE after the rep matmul (gates recip_rep copy)
      s_out:  +1 by DVE per normalized output tile (gates out DMA)
      s_done: +16 per out DMA (final drain wait)
    """
    import concourse.bass as bass
    import concourse.mybir as mybir

    f32 = mybir.dt.float32
    bf16 = mybir.dt.bfloat16
    FD = B_C + S_C  # fused free dim: 1024
    BUFS = 10

    nc = bass.Bass()
    gw_d = nc.dram_tensor("gw", [K_TILES, P, FD], bf16, kind="ExternalInput")
    plog_d = nc.dram_tensor("plog", [PAD_SLOTS, S_C], f32, kind="ExternalInput")
    out_d = nc.dram_tensor("out", [B_C, S_C], f32, kind="ExternalOutput")

    from contextlib import ExitStack

    with ExitStack() as ctx:
        gw_sb = ctx.enter_context(nc.sbuf_tensor([P, BUFS, FD], bf16))
        plog_sb = ctx.enter_context(nc.sbuf_tensor([PAD_SLOTS, S_C], f32))
        exp_sb = ctx.enter_context(nc.sbuf_tensor([PAD_SLOTS, S_C], f32))
        zero_col = ctx.enter_context(nc.sbuf_tensor([P, 1], f32))
        ones_col = ctx.enter_context(nc.sbuf_tensor([P, 1], f32))
        ones_row = ctx.enter_context(nc.sbuf_tensor([1, P], f32))
        recip_sb = ctx.enter_context(nc.sbuf_tensor([1, S_C], f32))
        recip_rep = ctx.enter_context(nc.sbuf_tensor([P, S_C], f32))
        o_sb = ctx.enter_context(nc.sbuf_tensor([P, M_TILES, S_C], f32))
        acc_ps = ctx.enter_context(nc.psum_tensor([P, M_TILES, S_C], f32))
        denom_ps = ctx.enter_context(nc.psum_tensor([1, S_C], f32))
        rep_ps = ctx.enter_context(nc.psum_tensor([P, S_C], f32))
        s_slot = [ctx.enter_context(nc.semaphore(name=f"s_slot{j}")) for j in range(BUFS)]
        s_plog = ctx.enter_context(nc.semaphore())
        s_mm = ctx.enter_context(nc.semaphore())
        s_init = ctx.enter_context(nc.semaphore())
        s_act = ctx.enter_context(nc.semaphore())
        s_den = ctx.enter_context(nc.semaphore())
        s_dve = ctx.enter_context(nc.semaphore())
        s_rep = ctx.enter_context(nc.semaphore())
        s_out = ctx.enter_context(nc.semaphore())
        s_done = ctx.enter_context(nc.semaphore())
        s_fin = ctx.enter_context(nc.semaphore())
        block = ctx.enter_context(nc.Block())

        @block.sync
        def _(sync):
            sync.dma_start(plog_sb[:], plog_d[:, :]).then_inc(s_plog, 16)
            for k in range(K_TILES):
                if k >= BUFS:
                    sync.wait_ge(s_mm, k - BUFS + 1)
                sync.dma_start(gw_sb[:, k % BUFS, :], gw_d[k, :, :]).then_inc(
                    s_slot[k % BUFS], 16
                )
            for m in range(M_TILES):
                sync.wait_ge(s_out, m + 1)
                sync.dma_start(
                    out_d[m * P : (m + 1) * P, :], o_sb[:, m, :]
                ).then_inc(s_done, 16)
            sync.wait_ge(s_done, 16 * M_TILES)

        @block.scalar
        def _(scalar):
            scalar.wait_ge(s_init, 1)
            scalar.wait_ge(s_plog, 16)
            scalar.activation(
                exp_sb[:],
                plog_sb[:],
                mybir.ActivationFunctionType.Exp,
                bias=zero_col[:],
            )
            scalar.activation(
                ones_col[:],
                plog_sb[:, 0:1],
                mybir.ActivationFunctionType.Copy,
                bias=1.0,
                scale=0.0,
            ).then_inc(s_act, 1)

        @block.tensor
        def _(tensor):
            for k in range(K_TILES):
                tensor.wait_ge(s_slot[k % BUFS], 16 * (k // BUFS + 1))
                tile = gw_sb[:, k % BUFS, :]
                for m in range(M_TILES):
                    mm = tensor.matmul(
                        acc_ps[:, m, :],
                        tile[:, m * P : (m + 1) * P],
                        tile[:, B_C:FD],
                        start=(k == 0),
                        stop=(k == K_TILES - 1),
                    )
                    if m == M_TILES - 1:
                        # rhs/lhsT fully streamed at retire -> safe to reuse
                        # the SBUF slot (write-back handled by drains below)
                        mm.then_inc(s_mm, 1)
                if k == 8:
                    tensor.wait_ge(s_act, 1)
                    tensor.matmul(
                        denom_ps[:], ones_col[:], exp_sb[:], start=True, stop=True
                    )
                    # drain flushes the PSUM writeback before consumers read
                    tensor.drain().then_inc(s_den, 1)
                elif k == 16:
                    tensor.wait_ge(s_dve, 1)
                    tensor.matmul(
                        rep_ps[:], ones_row[:], recip_sb[:], start=True, stop=True
                    )
                    tensor.drain().then_inc(s_rep, 1)
            tensor.drain().then_inc(s_fin, 1)

        @block.vector
        def _(vector):
            vector.memset(zero_col[:], 0.0)
            vector.memset(ones_row[:], 1.0).then_inc(s_init, 1)
            vector.wait_ge(s_den, 1)
            nc.vector.reciprocal(recip_sb[:], denom_ps[:]).then_inc(s_dve, 1)
            vector.wait_ge(s_rep, 1)
            nc.vector.tensor_copy(recip_rep[:], rep_ps[:])
            vector.wait_ge(s_fin, 1)
            for m in range(M_TILES):
                nc.vector.tensor_mul(
                    o_sb[:, m, :], acc_ps[:, m, :], recip_rep[:]
                ).then_inc(s_out, 1)

    nc.finalize()
    return nc


def _get_program():
    if "nc" not in _PROGRAM_CACHE:
        _PROGRAM_CACHE["nc"] = _build_program()
    return _PROGRAM_CACHE["nc"]


def _ensure_ntff_hook():
    """Make NTFF profiling under axon work (BASS_TRACE=1): the image's antenv
    package lacks the axon_hooks holder module, so synthesize it and register
    the ctypes-based profile hook from trn_agent_boot. Best-effort."""
    import types

    try:
        import antenv

        try:
            from antenv.axon_hooks import get_axon_ntff_profile_hook  # noqa: F401

            return  # already present and registered
        except ImportError:
            pass
        mod = types.ModuleType("antenv.axon_hooks")
        _holder = [None]
        mod.set_axon_ntff_profile_hook = lambda h: _holder.__setitem__(0, h)
        mod.get_axon_ntff_profile_hook = lambda: _holder[0]
        sys.modules["antenv.axon_hooks"] = mod
        antenv.axon_hooks = mod

        from trn_agent_boot.trn_boot import _ntff_profile_via_ctypes

        hook = _ntff_profile_via_ctypes("/opt/axon/libaxon_pjrt.so")
        mod.set_axon_ntff_profile_hook(hook)
    except Exception:
        pass


def kernel(**inputs):
    global LAST_RESULTS
    G = np.asarray(inputs["geneset_features"], dtype=np.float32)
    logits = np.asarray(inputs["attn_logits"], dtype=np.float32)
    flat_idx = np.asarray(inputs["flat_idx"]).astype(np.int64)
    seg = np.asarray(inputs["segment_ids"]).astype(np.int64)
    T = logits.shape[0]

    # Host-side layout prep: scatter exp(logits) into the sparse aggregation
    # matrix (member sets are sampled without replacement, so (idx, seg) pairs
    # are unique within a set and the fancy assignment is collision-free).
    e32 = np.exp(logits)
    W = np.zeros((NUM_GENESETS, NUM_SETS), dtype=ml_dtypes.bfloat16)
    W[flat_idx, seg] = e32.astype(ml_dtypes.bfloat16)

    # Padded per-set logit columns; device computes denominators from these.
    sizes = np.bincount(seg, minlength=NUM_SETS)
    starts = np.concatenate([[0], np.cumsum(sizes)[:-1]])
    pos = np.arange(T) - starts[seg]
    plogT = np.full((PAD_SLOTS, NUM_SETS), NEG_FILL, dtype=np.float32)
    plogT[pos, seg] = logits

    Gb = G.astype(ml_dtypes.bfloat16)

    GbT = np.ascontiguousarray(Gb.T)  # (8192, 1024)
    in_maps = []
    for c in range(N_CORES):
        bg, sg = divmod(c, SG)
        gt = GbT[:, bg * B_C : (bg + 1) * B_C].reshape(K_TILES, P, B_C)
        w = W[:, sg * S_C : (sg + 1) * S_C].reshape(K_TILES, P, S_C)
        gw = np.concatenate([gt, w], axis=2)  # (K_TILES, P, B_C + S_C)
        plog = np.ascontiguousarray(plogT[:, sg * S_C : (sg + 1) * S_C])
        in_maps.append({"gw": np.ascontiguousarray(gw), "plog": plog})

    from concourse.bass_utils import run_bass_kernel_spmd

    _ensure_ntff_hook()
    nc = _get_program()
    res = run_bass_kernel_spmd(nc, in_maps, core_ids=list(range(N_CORES)))
    LAST_RESULTS = res

    out = np.empty((BATCH, NUM_SETS), dtype=np.float32)
    for c in range(N_CORES):
        bg, sg = divmod(c, SG)
        out[bg * B_C : (bg + 1) * B_C, sg * S_C : (sg + 1) * S_C] = res.results[c][
            "out"
        ]
    return out
